# revision 1
# baseline (speedup 1.0000x reference)
"""Trainium2 Bass kernel for causal FFT convolution (nn_CausalConvolution).

y = irfft(rfft(bf16(x), 2T) * rfft(h, 2T))[..., :T],  x,h: (8, 64, 65536) fp32.

Identity used: with z = bf16(x) + i*h,  y = Im(iFFT(FFT_2T(z)^2)) / 2.
One complex forward + one complex inverse FFT per channel, N = 131072,
decomposed as radix (128, 128, 8) matmul stages on the PE with PE-transposes
between stages. 512 channels sharded 64-per-core across 8 NeuronCores (pure
data parallelism). See mirror layouts in the stage comments.

Self-contained: shapes/sharding hardcoded; tables computed with numpy here.
"""
import numpy as np
import ml_dtypes
from contextlib import ExitStack

import concourse.bass as bass
import concourse.bacc as bacc
import concourse.tile as tile
import concourse.mybir as mybir
from concourse.bass_utils import run_bass_kernel_spmd

F32 = mybir.dt.float32
F32R = mybir.dt.float32r
BF16 = mybir.dt.bfloat16
MUL = mybir.AluOpType.mult
ADD = mybir.AluOpType.add
SUB = mybir.AluOpType.subtract

Bsz, Csz, T = 8, 64, 65536
NFFT = 2 * T
NCORES = 8
CPC = (Bsz * Csz) // NCORES          # 64 channels per core
NBLK = CPC // 2                      # 2 channels per block

_Wc = lambda M, E: np.exp(-2j * np.pi * E / M)


def _gen_tables():
    F128 = _Wc(128, np.outer(np.arange(128), np.arange(128)))
    F8 = _Wc(8, np.outer(np.arange(8), np.arange(8)))
    W1024bd = _Wc(1024, np.outer(np.arange(8), np.arange(128)))     # [b, d]
    TW1_cab = _Wc(NFFT, (8 * np.arange(128)[None, :, None]
                         + np.arange(8)[None, None, :])
                  * np.arange(128)[:, None, None])                  # [c, a, b]

    f32 = lambda v: np.ascontiguousarray(v, dtype=np.float32)
    t = {}
    # ---- S1 (contract u<64, produce c): lhsT[u, c] ----
    t["s1_re"] = f32(F128[:64].real)
    t["s1_im"] = f32(F128[:64].imag)
    t["s1_imn"] = f32(-F128[:64].imag)

    # ---- packed [n,128,128] f32r stationaries ----
    mats = []
    idx = {}

    def put(name, m):
        idx[name] = len(mats)
        mats.append(f32(m))

    S2 = F128[None, :, :] * W1024bd[:, None, :]                     # [b, a, d]
    for b in range(8):
        put(f"s2_re{b}", S2[b].real)
        put(f"s2_im{b}", S2[b].imag)
        put(f"s2_imn{b}", -S2[b].imag)
    S3 = np.zeros((128, 128), np.complex128)
    for b in range(8):
        for e in range(8):
            for c16 in range(16):
                S3[c16 * 8 + b, c16 * 8 + e] = F8[b, e]
    put("s3_re", S3.real)
    put("s3_im", S3.imag)
    put("s3_imn", -S3.imag)
    S3p = np.zeros((128, 128), np.complex128)
    for e in range(8):
        for bp in range(8):
            for c16 in range(16):
                S3p[c16 * 8 + e, c16 * 8 + bp] = np.conj(F8[bp, e])
    put("s3p_re", S3p.real)          # multiplies rhs_re -> psum_re
    put("s3p_im", S3p.imag)          # rhs_re -> psum_im
    put("s3p_imn2", -2 * S3p.imag)   # rhs_im -> psum_re (x2: Sim stored halved)
    put("s3p_re2", 2 * S3p.real)     # rhs_im -> psum_im
    S2p = np.conj(S2).transpose(0, 2, 1)                            # [bp, d, a]
    for b in range(8):
        put(f"s2p_re{b}", S2p[b].real)
        put(f"s2p_im{b}", S2p[b].imag)
        put(f"s2p_imn{b}", -S2p[b].imag)
    put("ident", np.eye(128))
    t["st128"] = np.stack(mats)                                     # [n,128,128]
    t["st128_idx"] = idx

    # ---- S1' (contract c, produce u<64, imag plane only, scale 1/(2N)) ----
    S1p = np.conj(F128).T[:, :64] / (2.0 * NFFT)                    # [c, u]
    t["s1p_re"] = f32(S1p.real)
    t["s1p_im"] = f32(S1p.imag)

    # ---- twiddle tables ----
    m_ord = TW1_cab.reshape(128, 1024)
    tw1 = np.concatenate([m_ord, m_ord], axis=1)                    # [c, 2048]
    tw1p = np.zeros((128, 2048), np.complex128)                     # [a, bp*256+ch*128+c]
    for bp in range(8):
        for ch in range(2):
            sl = slice(bp * 256 + ch * 128, bp * 256 + ch * 128 + 128)
            tw1p[:, sl] = np.conj(TW1_cab[:, :, bp]).T
    t["tw"] = np.concatenate(
        [f32(m_ord.real), f32(m_ord.imag),
         f32(tw1p.real), f32(tw1p.imag)], axis=1)                   # [128, 6144]
    return t


def _build(n_blocks=NBLK, debug=False):
    tabs = _gen_tables()
    nc = bacc.Bacc("TRN2", target_bir_lowering=False, debug=False)

    x_d = nc.dram_tensor("x_in", [CPC, 64, 1024], BF16, kind="ExternalInput").ap()
    h_d = nc.dram_tensor("h_in", [CPC, 64, 1024], F32R, kind="ExternalInput").ap()
    nst = tabs["st128"].shape[0]
    st_d = nc.dram_tensor("st_in", [nst, 128, 128], F32R, kind="ExternalInput").ap()
    s1bf_d = nc.dram_tensor("s1bf_in", [2, 64, 128], BF16, kind="ExternalInput").ap()
    s1fr_d = nc.dram_tensor("s1fr_in", [2, 64, 128], F32R, kind="ExternalInput").ap()
    s1p_d = nc.dram_tensor("s1p_in", [2, 128, 64], F32R, kind="ExternalInput").ap()
    tw_d = nc.dram_tensor("tw_in", [128, 6144], F32, kind="ExternalInput").ap()
    y_d = nc.dram_tensor("y_out", [CPC, 64, 1024], F32, kind="ExternalOutput").ap()
    dbg_d = {}
    if debug:
        for nm in ["B1", "B2", "B3", "B4", "B5", "B6", "B7", "B8", "B9"]:
            for pl in ["re", "im"]:
                dbg_d[nm + pl] = nc.dram_tensor(
                    f"dbg_{nm}{pl}", [128, 2048], F32, kind="ExternalOutput").ap()

    with tile.TileContext(nc) as tc, ExitStack() as ctx:
        const = ctx.enter_context(tc.tile_pool(name="const", bufs=1))
        data = ctx.enter_context(tc.tile_pool(name="io", bufs=2))
        stage = ctx.enter_context(tc.tile_pool(name="stage", bufs=4))
        stageB = ctx.enter_context(tc.tile_pool(name="stageB", bufs=3))
        tmp = ctx.enter_context(tc.tile_pool(name="tmp", bufs=2))
        psum = ctx.enter_context(tc.tile_pool(name="psum", bufs=4, space="PSUM"))

        # ---- load constant tables once ----
        st = const.tile([128, nst * 128], F32R, tag="st")
        nc.sync.dma_start(
            st[:].rearrange("p (n c) -> p n c", n=nst),
            st_d.rearrange("n p c -> p n c"))
        s1bf = const.tile([64, 2 * 128], BF16, tag="s1bf")
        nc.sync.dma_start(s1bf[:].rearrange("p (n c) -> p n c", n=2),
                          s1bf_d.rearrange("n p c -> p n c"))
        s1fr = const.tile([64, 2 * 128], F32R, tag="s1fr")
        nc.sync.dma_start(s1fr[:].rearrange("p (n c) -> p n c", n=2),
                          s1fr_d.rearrange("n p c -> p n c"))
        s1p = const.tile([128, 2 * 64], F32R, tag="s1p")
        nc.sync.dma_start(s1p[:].rearrange("p (n c) -> p n c", n=2),
                          s1p_d.rearrange("n p c -> p n c"))
        tw = const.tile([128, 6144], F32, tag="tw")
        nc.sync.dma_start(tw[:], tw_d)

        sidx = tabs["st128_idx"]
        M = lambda name: st[:, sidx[name] * 128:(sidx[name] + 1) * 128]
        ident = M("ident")
        s1bf_re, s1bf_im = s1bf[:, 0:128], s1bf[:, 128:256]
        s1fr_imn, s1fr_re = s1fr[:, 0:128], s1fr[:, 128:256]
        s1p_re, s1p_im = s1p[:, 0:64], s1p[:, 64:128]
        tw1_re, tw1_im = tw[:, 0:1024], tw[:, 1024:2048]
        tw1p_re, tw1p_im = tw[:, 2048:4096], tw[:, 4096:6144]

        def cmm(pre, pim, mrr, mir, mri, mii, rre, rim, start, stop):
            """pre += mrr.T@rre + mir.T@rim ; pim += mri.T@rre + mii.T@rim"""
            nc.tensor.matmul(pre, mrr, rre, start=start, stop=False)
            nc.tensor.matmul(pre, mir, rim, start=False, stop=stop)
            nc.tensor.matmul(pim, mri, rre, start=start, stop=False)
            nc.tensor.matmul(pim, mii, rim, start=False, stop=stop)

        def pair(dt=F32):
            pr = psum.tile([128, 512], dt, tag="pr")
            pi = psum.tile([128, 512], dt, tag="pi")
            return pr, pi

        def cmul_ev(pre, pim, twre, twim, ore, oim):
            """(ore,oim) = (pre+i*pim) * (twre+i*twim), 512-wide."""
            t1 = tmp.tile([128, 512], F32, tag="t1")
            t2 = tmp.tile([128, 512], F32, tag="t2")
            nc.vector.tensor_tensor(t1[:], pre, twre, MUL)
            nc.vector.tensor_tensor(t2[:], pim, twim, MUL)
            nc.vector.tensor_tensor(ore, t1[:], t2[:], SUB)
            nc.vector.tensor_tensor(t1[:], pre, twim, MUL)
            nc.vector.tensor_tensor(t2[:], pim, twre, MUL)
            nc.vector.tensor_tensor(oim, t1[:], t2[:], ADD)

        def dbg_tap(name, tre, tim):
            if debug:
                nc.sync.dma_start(dbg_d[name + "re"][:], tre[:].bitcast(F32))
                nc.sync.dma_start(dbg_d[name + "im"][:], tim[:].bitcast(F32))

        for blk in range(n_blocks):
            ch0 = 2 * blk
            # ---- load: xq (bf16 quantize via gpsimd cast-dma), h (f32r) ----
            xq = data.tile([64, 2048], BF16, tag="xq")
            l0im = data.tile([64, 2048], F32R, tag="l0im")
            for ch in range(2):
                nc.sync.dma_start(xq[:, ch * 1024:(ch + 1) * 1024], x_d[ch0 + ch])
                nc.sync.dma_start(l0im[:, ch * 1024:(ch + 1) * 1024], h_d[ch0 + ch])

            # ---- S1 + EV1(TW1): B1 [c x (ch*1024 + a*8 + b)] ----
            b1re = stage.tile([128, 2048], F32R, tag="pAre")
            b1im = stage.tile([128, 2048], F32R, tag="pAim")
            for ck in range(4):            # 512-chunks of (ch*1024 + m)
                cs = slice(ck * 512, (ck + 1) * 512)
                pr, pi = pair()
                cmm(pr[:], pi[:], s1bf_re, s1fr_imn, s1bf_im, s1fr_re,
                    xq[:, cs], l0im[:, cs], True, True)
                ts_ = slice((ck % 2) * 512, (ck % 2) * 512 + 512)
                cmul_ev(pr[:], pi[:], tw1_re[:, ts_], tw1_im[:, ts_],
                        b1re[:, cs], b1im[:, cs])
            dbg_tap("B1", b1re, b1im)

            # ---- TR1 + EV2: B2 [a x (b*256 + ch*128 + c)] ----
            b2re = stageB.tile([128, 2048], F32R, tag="pBre")
            b2im = stageB.tile([128, 2048], F32R, tag="pBim")
            b1v_re = b1re[:].rearrange("p (ch a b) -> p ch a b", ch=2, a=128, b=8)
            b1v_im = b1im[:].rearrange("p (ch a b) -> p ch a b", ch=2, a=128, b=8)
            b2v_re = b2re[:].rearrange("p (b ch c) -> p b ch c", b=8, ch=2, c=128)
            b2v_im = b2im[:].rearrange("p (b ch c) -> p b ch c", b=8, ch=2, c=128)
            for ch in range(2):
                for hb in range(2):        # b half: 4 transposes per psum tile
                    pr, pi = pair(F32R)
                    for j in range(4):
                        b = hb * 4 + j
                        s = slice(j * 128, (j + 1) * 128)
                        nc.tensor.transpose(pr[:, s], b1v_re[:, ch, :, b], ident)
                        nc.tensor.transpose(pi[:, s], b1v_im[:, ch, :, b], ident)
                    for ps, ov in ((pr, b2v_re), (pi, b2v_im)):
                        nc.scalar.copy(
                            ov[:, hb * 4:(hb + 1) * 4, ch, :],
                            ps[:].rearrange("p (j c) -> p j c", j=4))
            dbg_tap("B2", b2re, b2im)

            # ---- S2 + EV3: B3 [d x (ch*1024 + c*8 + b)] ----
            b3re = stage.tile([128, 2048], F32R, tag="pAre")
            b3im = stage.tile([128, 2048], F32R, tag="pAim")
            b3v_re = b3re[:].rearrange("p (ch c b) -> p ch c b", ch=2, c=128, b=8)
            b3v_im = b3im[:].rearrange("p (ch c b) -> p ch c b", ch=2, c=128, b=8)
            for hb in range(4):            # 2 b per psum pair
                pr, pi = pair()
                for j in range(2):
                    b = hb * 2 + j
                    s = slice(j * 256, (j + 1) * 256)
                    rs = slice(b * 256, (b + 1) * 256)
                    cmm(pr[:, s], pi[:, s],
                        M(f"s2_re{b}"), M(f"s2_imn{b}"), M(f"s2_im{b}"), M(f"s2_re{b}"),
                        b2re[:, rs], b2im[:, rs], True, True)
                for ps, ov in ((pr, b3v_re), (pi, b3v_im)):
                    iv = ps[:].rearrange("p (j ch c) -> p j ch c", j=2, ch=2)
                    for ch in range(2):
                        nc.scalar.copy(
                            ov[:, ch, :, hb * 2:(hb + 1) * 2]
                            .rearrange("p c j -> p j c"),
                            iv[:, :, ch, :])
            dbg_tap("B3", b3re, b3im)

            # ---- TR2 + EV4: B4 [(c16*8+b) x (ch*1024 + chi*128 + d)] ----
            b4re = stageB.tile([128, 2048], F32R, tag="pBre")
            b4im = stageB.tile([128, 2048], F32R, tag="pBim")
            for ch in range(2):
                for hc in range(2):        # chi half
                    pr, pi = pair(F32R)
                    for j in range(4):
                        chi = hc * 4 + j
                        s = slice(j * 128, (j + 1) * 128)
                        src = slice(ch * 1024 + chi * 128, ch * 1024 + (chi + 1) * 128)
                        nc.tensor.transpose(pr[:, s], b3re[:, src], ident)
                        nc.tensor.transpose(pi[:, s], b3im[:, src], ident)
                    ds = slice(ch * 1024 + hc * 512, ch * 1024 + (hc + 1) * 512)
                    nc.scalar.copy(b4re[:, ds], pr[:])
                    nc.scalar.copy(b4im[:, ds], pi[:])
            dbg_tap("B4", b4re, b4im)

            # ---- S3 + EV5(square): B5 = (Sre, Sim/2) ----
            b5re = stage.tile([128, 2048], F32R, tag="pAre")
            b5im = stage.tile([128, 2048], F32R, tag="pAim")
            for ck in range(4):
                cs = slice(ck * 512, (ck + 1) * 512)
                pr, pi = pair()
                cmm(pr[:], pi[:], M("s3_re"), M("s3_imn"), M("s3_im"), M("s3_re"),
                    b4re[:, cs], b4im[:, cs], True, True)
                sqre = tmp.tile([128, 512], F32, tag="t1")
                sqim = tmp.tile([128, 512], F32, tag="t2")
                zim = tmp.tile([128, 512], F32, tag="t3")
                nc.scalar.activation(sqre[:], pr[:], mybir.ActivationFunctionType.Square)
                nc.scalar.activation(sqim[:], pi[:], mybir.ActivationFunctionType.Square)
                nc.scalar.copy(zim[:], pi[:])
                nc.vector.tensor_tensor(b5re[:, cs], sqre[:], sqim[:], SUB)
                nc.vector.tensor_tensor(b5im[:, cs], pr[:], zim[:], MUL)
            dbg_tap("B5", b5re, b5im)

            # ---- S3' + EV6: B6 [(c16*8+bp) x cols] ----
            b6re = stageB.tile([128, 2048], F32R, tag="pBre")
            b6im = stageB.tile([128, 2048], F32R, tag="pBim")
            for ck in range(4):
                cs = slice(ck * 512, (ck + 1) * 512)
                pr, pi = pair()
                cmm(pr[:], pi[:], M("s3p_re"), M("s3p_imn2"), M("s3p_im"), M("s3p_re2"),
                    b5re[:, cs], b5im[:, cs], True, True)
                nc.scalar.copy(b6re[:, cs], pr[:])
                nc.scalar.copy(b6im[:, cs], pi[:])
            dbg_tap("B6", b6re, b6im)

            # ---- TR3 + EV7: B7 [d x (bp*256 + ch*128 + c)] ----
            b7re = stage.tile([128, 2048], F32R, tag="pAre")
            b7im = stage.tile([128, 2048], F32R, tag="pAim")
            b7v_re = b7re[:].rearrange("p (bp ch chi c16) -> p bp ch chi c16",
                                       bp=8, ch=2, chi=8, c16=16)
            b7v_im = b7im[:].rearrange("p (bp ch chi c16) -> p bp ch chi c16",
                                       bp=8, ch=2, chi=8, c16=16)
            for ch in range(2):
                for hc in range(2):
                    pr, pi = pair(F32R)
                    for j in range(4):
                        chi = hc * 4 + j
                        s = slice(j * 128, (j + 1) * 128)
                        src = slice(ch * 1024 + chi * 128, ch * 1024 + (chi + 1) * 128)
                        nc.tensor.transpose(pr[:, s], b6re[:, src], ident)
                        nc.tensor.transpose(pi[:, s], b6im[:, src], ident)
                    for ps, ov in ((pr, b7v_re), (pi, b7v_im)):
                        iv = ps[:].rearrange("p (chi c16 bp) -> p chi c16 bp",
                                             chi=4, c16=16, bp=8)
                        eng = nc.vector if (ch == 0) else nc.scalar
                        if ch == 0:
                            nc.vector.tensor_copy(
                                ov[:, :, ch, hc * 4:(hc + 1) * 4, :]
                                .rearrange("p bp chi c16 -> p chi c16 bp"), iv[:])
                        else:
                            nc.scalar.copy(
                                ov[:, :, ch, hc * 4:(hc + 1) * 4, :]
                                .rearrange("p bp chi c16 -> p chi c16 bp"), iv[:])
            dbg_tap("B7", b7re, b7im)

            # ---- S2' + EV8(TW1'): B8 [a x (bp*256 + ch*128 + c)] ----
            b8re = stageB.tile([128, 2048], F32R, tag="pBre")
            b8im = stageB.tile([128, 2048], F32R, tag="pBim")
            for hb in range(4):
                pr, pi = pair()
                for j in range(2):
                    b = hb * 2 + j
                    s = slice(j * 256, (j + 1) * 256)
                    rs = slice(b * 256, (b + 1) * 256)
                    cmm(pr[:, s], pi[:, s],
                        M(f"s2p_re{b}"), M(f"s2p_imn{b}"), M(f"s2p_im{b}"), M(f"s2p_re{b}"),
                        b7re[:, rs], b7im[:, rs], True, True)
                cs = slice(hb * 512, (hb + 1) * 512)
                cmul_ev(pr[:], pi[:], tw1p_re[:, cs], tw1p_im[:, cs],
                        b8re[:, cs], b8im[:, cs])
            dbg_tap("B8", b8re, b8im)

            # ---- TR4 + EV9: B9 [c x (ch*1024 + a*8 + bp)] ----
            b9re = stage.tile([128, 2048], F32R, tag="pAre")
            b9im = stage.tile([128, 2048], F32R, tag="pAim")
            b8v_re = b8re[:].rearrange("p (bp ch c) -> p bp ch c", bp=8, ch=2, c=128)
            b8v_im = b8im[:].rearrange("p (bp ch c) -> p bp ch c", bp=8, ch=2, c=128)
            b9v_re = b9re[:].rearrange("p (ch a bp) -> p ch a bp", ch=2, a=128, bp=8)
            b9v_im = b9im[:].rearrange("p (ch a bp) -> p ch a bp", ch=2, a=128, bp=8)
            for ch in range(2):
                for hb in range(2):
                    pr, pi = pair(F32R)
                    for j in range(4):
                        bp = hb * 4 + j
                        s = slice(j * 128, (j + 1) * 128)
                        nc.tensor.transpose(pr[:, s], b8v_re[:, bp, ch, :], ident)
                        nc.tensor.transpose(pi[:, s], b8v_im[:, bp, ch, :], ident)
                    for ps, ov in ((pr, b9v_re), (pi, b9v_im)):
                        nc.scalar.copy(
                            ov[:, ch, :, hb * 4:(hb + 1) * 4]
                            .rearrange("p a j -> p j a"),
                            ps[:].rearrange("p (j a) -> p j a", j=4))
            dbg_tap("B9", b9re, b9im)

            # ---- S1' + EV10 + store ----
            for ch in range(2):
                for q in range(2):
                    p10 = psum.tile([64, 512], F32, tag="pr")
                    rs = slice(ch * 1024 + q * 512, ch * 1024 + (q + 1) * 512)
                    nc.tensor.matmul(p10[:], s1p_im, b9re[:, rs], start=True, stop=False)
                    nc.tensor.matmul(p10[:], s1p_re, b9im[:, rs], start=False, stop=True)
                    yt = data.tile([64, 512], F32, tag="yt")
                    nc.scalar.copy(yt[:], p10[:])
                    nc.sync.dma_start(
                        y_d[ch0 + ch].rearrange("u (q m) -> u q m", q=2)[:, q, :],
                        yt[:])

    nc.compile()
    return nc, tabs


_CACHE = {}


def _get(n_blocks=NBLK, debug=False):
    key = (n_blocks, debug)
    if key not in _CACHE:
        _CACHE[key] = _build(n_blocks, debug)
    return _CACHE[key]


def _in_maps(x, h, tabs):
    xf = np.ascontiguousarray(x, np.float32).reshape(Bsz * Csz, 65536)
    hf = np.ascontiguousarray(h, np.float32).reshape(Bsz * Csz, 65536)
    s1bf = np.stack([
        tabs["s1_re"].astype(ml_dtypes.bfloat16),
        tabs["s1_im"].astype(ml_dtypes.bfloat16)])
    s1fr = np.stack([tabs["s1_imn"], tabs["s1_re"]])
    s1p = np.stack([tabs["s1p_re"], tabs["s1p_im"]])
    maps = []
    for i in range(NCORES):
        sl = slice(i * CPC, (i + 1) * CPC)
        maps.append({
            "x_in": xf[sl].reshape(CPC, 64, 1024).astype(ml_dtypes.bfloat16),
            "h_in": hf[sl].reshape(CPC, 64, 1024),
            "st_in": tabs["st128"],
            "s1bf_in": s1bf,
            "s1fr_in": s1fr,
            "s1p_in": s1p,
            "tw_in": tabs["tw"],
        })
    return maps


def kernel(x, h):
    nc, tabs = _get()
    maps = _in_maps(x, h, tabs)
    res = run_bass_kernel_spmd(nc, maps, core_ids=list(range(NCORES)))
    y = np.concatenate([r["y_out"].reshape(CPC, 65536) for r in res.results])
    return y.reshape(Bsz, Csz, T).astype(np.float32)

